# revision 46
# baseline (speedup 1.0000x reference)
"""ExpLeak (leaky integrator) Trainium2 kernel.

Computes, over a [B=16, T=1024, N=4096] f32 tensor:
    y[b, t, n] = alpha * y[b, t-1, n] + x[b, t, n],   alpha = exp(-1/tau)

Strategy
--------
Pure data parallel over batch: 8 NeuronCores x 2 batches each.

Per core, the time recurrence is evaluated as a blocked lower-triangular
matmul.  For a time chunk of C=128 steps,

    y_chunk = L @ x_chunk + alphas (x) carry          (outer product)
    L[t, s]    = alpha^(t-s)  for s <= t, else 0
    alphas[t]  = alpha^(t+1)
    carry[n]   = y[last row of previous chunk, n]

Both terms are PE matmuls accumulating into the same PSUM bank:
  - main:  lhsT = L^T  [128,128], rhs = x tile slice [128, 512]
  - carry: lhsT = alphas [1,128], rhs = carry row    [1,   512]  (K=1)
The carry row for the next chunk is PSUM row 127, moved to partition 0
of an SBUF tile with a small DMA.  float32r matmuls (full-rate fp32 on
the PE) keep the PE far from the HBM roofline (the kernel is
memory-bound: 64 MiB of HBM traffic per core).
"""

import os
import sys

import numpy as np


def _ensure_concourse():
    try:
        import concourse.bass  # noqa: F401
        return
    except ImportError:
        pass
    for p in ("/opt/trn_rl_repo", "/root/.axon_site/_ro/trn_rl_repo"):
        if os.path.isdir(p) and p not in sys.path:
            sys.path.insert(0, p)
    import concourse.bass  # noqa: F401


B, T, N = 16, 1024, 4096
N_CORES = 8
B_PER = B // N_CORES  # batches per core
C = 128               # time chunk (PE contraction dim)
NCHUNK = T // C
FT = 512              # feature tile (max fp32 moving free dim / PSUM bank)
NFT = N // FT

_PROGRAM_CACHE = {}


def build_program():
    """Trace + compile the per-core Bass/Tile program. alpha enters only
    through the lt/av input tensors, so one program serves any tau."""
    _ensure_concourse()
    import concourse.bacc as bacc
    import concourse.mybir as mybir
    from concourse import tile

    DT = mybir.dt.float32
    DTR = mybir.dt.float32r

    nc = bacc.Bacc("TRN2", target_bir_lowering=False, debug=False,
                   num_devices=N_CORES)
    x = nc.declare_dram_parameter("x", [B_PER, T, N], DT, isOutput=False)
    lt = nc.declare_dram_parameter("lt", [C, C], DT, isOutput=False)
    ltl = nc.declare_dram_parameter("ltl", [C, C], DT, isOutput=False)
    av = nc.declare_dram_parameter("av", [1, C], DT, isOutput=False)
    y = nc.declare_dram_parameter("y", [B_PER, T, N], DT, isOutput=True)

    with tile.TileContext(nc) as tc:
        with (
            tc.tile_pool(name="w", bufs=1) as wpool,
            tc.tile_pool(name="xp", bufs=4) as xpool,
            tc.tile_pool(name="op", bufs=3) as opool,
            tc.tile_pool(name="cp", bufs=2) as cpool,
            tc.tile_pool(name="ps", bufs=8, space="PSUM") as pspool,
        ):
            # fp32r tiles: the PE reads the top 20 bits (e8m11); the DMA
            # just moves fp32 bits, so PE input is the truncation of the
            # fp32 value (~1.2e-4 rms).  Weights are pre-rounded on host.
            # L^T is split Dekker-style into hi+lo fp32r parts so the
            # main-matmul weights are exact to fp32.
            ltt = wpool.tile([C, C], DTR, tag="lt")
            nc.sync.dma_start(ltt[:], lt[:].bitcast(DTR))
            ltlt = wpool.tile([C, C], DTR, tag="ltl")
            nc.sync.dma_start(ltlt[:], ltl[:].bitcast(DTR))
            avt = wpool.tile([1, C], DTR, tag="av")
            nc.sync.dma_start(avt[:], av[:].bitcast(DTR))

            carry = {}
            for k in range(NCHUNK):
                trange = slice(k * C, (k + 1) * C)
                for b in range(B_PER):
                    xt = xpool.tile([C, N], DTR, tag="xt")
                    nc.sync.dma_start(xt[:], x[b, trange, :].bitcast(DTR))
                    ot = opool.tile([C, N], DT, tag="ot")
                    newcarry = cpool.tile([1, N], DTR, tag="carry")
                    for j in range(NFT):
                        fsl = slice(j * FT, (j + 1) * FT)
                        ps = pspool.tile([C, FT], DT, tag="ps")
                        nc.tensor.matmul(
                            ps[:],
                            ltt[:],
                            xt[:, fsl],
                            start=True,
                            stop=False,
                        )
                        nc.tensor.matmul(
                            ps[:],
                            ltlt[:],
                            xt[:, fsl],
                            start=False,
                            stop=(k == 0),
                        )
                        if k > 0:
                            nc.tensor.matmul(
                                ps[:],
                                avt[:],
                                carry[b][0:1, fsl],
                                start=False,
                                stop=True,
                            )
                        nc.vector.tensor_copy(ot[:, fsl], ps[:])
                    # next chunk's carry: out row 127 -> partition 0 (the
                    # PE rounds the fp32 bits to fp32r on read).  SWDGE
                    # (gpsimd) keeps this dependent little DMA out of the
                    # HWDGE FIFOs (no head-of-line blocking).
                    nc.gpsimd.dma_start(newcarry[0:1, :],
                                        ot[C - 1:C, :].bitcast(DTR))
                    # stores ride the ACT HWDGE ring so the SP ring only
                    # carries loads and can stream ahead.
                    nc.scalar.dma_start(y[b, trange, :], ot[:])
                    carry[b] = newcarry

    nc.compile()
    return nc


def _get_program():
    nc = _PROGRAM_CACHE.get("nc")
    if nc is None:
        nc = build_program()
        _PROGRAM_CACHE["nc"] = nc
    return nc


def _round_fp32r(a: np.ndarray) -> np.ndarray:
    """Round fp32 to the PE's fp32r grid (e8m11: low 12 mantissa bits
    zero), round-to-nearest-even."""
    bits = a.astype(np.float32).view(np.uint32)
    keep = np.uint32(0xFFFFF000)
    low = bits & np.uint32(0xFFF)
    lsb = (bits >> np.uint32(12)) & np.uint32(1)
    round_up = (low > 0x800) | ((low == 0x800) & (lsb == 1))
    out = (bits & keep) + np.where(round_up, np.uint32(0x1000), np.uint32(0))
    return out.view(np.float32)


def make_weights(alpha: float):
    """Host-side constant tensors, all on the fp32r grid:
    lt/ltl = hi/lo Dekker split of L^T (upper triangular in (s,t));
    av[0,t] = alpha^(t+1), bias-compensated for carry truncation."""
    powers = np.power(np.float64(alpha), np.arange(C + 1))
    lt = np.zeros((C, C), dtype=np.float32)
    s_idx, t_idx = np.meshgrid(np.arange(C), np.arange(C), indexing="ij")
    mask = s_idx <= t_idx
    lt[mask] = powers[(t_idx - s_idx)[mask]].astype(np.float32)
    av = powers[1:].astype(np.float32).reshape(1, C)
    lt_hi = _round_fp32r(lt)
    lt_lo = _round_fp32r((lt - lt_hi).astype(np.float32))
    return lt_hi, lt_lo, _round_fp32r(av)


def kernel(input_current: np.ndarray, tau_mem: np.ndarray) -> np.ndarray:
    _ensure_concourse()
    from concourse.bass_utils import run_bass_kernel_spmd

    # Pre-round x to the fp32r grid (round-to-nearest instead of the
    # PE's truncation of the low 12 bits: halves the input error).
    x = _round_fp32r(np.ascontiguousarray(input_current, dtype=np.float32))
    tau = np.float32(np.asarray(tau_mem).reshape(-1)[0])
    alpha = float(np.exp(np.float32(-1.0) / tau))
    lt_hi, lt_lo, av1 = make_weights(alpha)

    nc = _get_program()
    in_maps = [
        {"x": x[c * B_PER:(c + 1) * B_PER], "lt": lt_hi, "ltl": lt_lo,
         "av": av1}
        for c in range(N_CORES)
    ]
    res = run_bass_kernel_spmd(nc, in_maps, list(range(N_CORES)))
    out = np.concatenate([res.results[c]["y"] for c in range(N_CORES)], axis=0)
    return out.astype(np.float32, copy=False)
